# revision 2
# baseline (speedup 1.0000x reference)
"""NodeUnpool kernel for 8 Trainium2 NeuronCores (Bass/Tile, SPMD).

Computation (see nn.Module reference):
    old = h_full[old_idxs]                      # [M, 256] gather
    merged = old @ W1.T + b1 + h_sub @ W2.T + b2
    out = h_full with rows old_idxs replaced by merged

Strategy:
  * old_idxs is arange(M) in this problem (fill="arange"), so the gather and
    scatter are contiguous row slices. A general host-side gather/scatter
    fallback handles any other index pattern.
  * The device work is exactly the merged-row GEMM: X=[old | h_sub] [M,512]
    @ Wc.T + (b1+b2), sharded row-wise across 8 cores (M/8 = 31250 rows each).
  * Activations are fed feature-major (host pre-transpose) so the 512-deep
    contraction lies on SBUF partitions: outT[j,r] = sum_k Wc.T[k,j] * X.T[k,r].
    Weights are the stationary operand; PSUM accumulates 4 k-tiles; the bias is
    added during PSUM->SBUF eviction (per-partition scalar on the DVE).
  * float32r (bit-identical fp32 storage, relaxed-precision PE path) runs the
    PE at 4x the fp32 matmul rate, keeping the kernel DMA-bound as intended
    for this memory-regime problem. Max rel-err vs fp32 ~1e-4.
  * Pass-through rows (h_full[M:]) never touch the device; they are copied on
    the host during output assembly.
"""

import sys
from concurrent.futures import ThreadPoolExecutor

import numpy as np

N, M, DIM = 1_000_000, 250_000, 256
N_CORES = 8
ROWS_PC = M // N_CORES  # 31250 merged rows per core
CHUNK = 2048            # columns (rows of X) processed per inner step
R_PAD = 31744           # 15*2048 + 1024, first multiple-of-512 tiling >= 31250
KT = (2 * DIM) // 128   # 4 contraction tiles
JT = DIM // 128         # 2 output-feature blocks

_NC_CACHE = {}
_POOL = ThreadPoolExecutor(max_workers=N_CORES)


def _ensure_concourse():
    try:
        import concourse.bass  # noqa: F401
    except ImportError:  # pragma: no cover
        sys.path.insert(0, "/opt/trn_rl_repo")
        import concourse.bass  # noqa: F401


def _build_nc(in_dt="float32r", repeat=1):
    """Build + bacc-compile the per-core Bass program (identical on all cores).

    repeat>1 re-runs the steady-state column loop that many times inside one
    NEFF (idempotent writes) — used by the slope-based HW timing harness.
    """
    _ensure_concourse()
    import concourse.bacc as bacc
    import concourse.tile as tile
    from concourse import mybir

    dt_in = getattr(mybir.dt, in_dt)
    f32 = mybir.dt.float32

    nc = bacc.Bacc("TRN2", target_bir_lowering=False, debug=False)
    xT = nc.dram_tensor("xT", [2 * DIM, R_PAD], dt_in, kind="ExternalInput")
    wT = nc.dram_tensor("wT", [2 * DIM, DIM], dt_in, kind="ExternalInput")
    bias = nc.dram_tensor("bias", [128, JT], f32, kind="ExternalInput")
    outT = nc.dram_tensor("outT", [DIM, R_PAD], f32, kind="ExternalOutput")

    with tile.TileContext(nc) as tc:
        with (
            tc.tile_pool(name="wpool", bufs=1) as wpool,
            tc.tile_pool(name="io", bufs=2) as io,
            tc.tile_pool(name="pp", bufs=4, space="PSUM") as pp,
        ):
            w_sb = wpool.tile([128, KT * DIM], dt_in)
            for kt in range(KT):
                nc.sync.dma_start(
                    out=w_sb[:, kt * DIM : (kt + 1) * DIM],
                    in_=wT[kt * 128 : (kt + 1) * 128, :],
                )
            b_sb = wpool.tile([128, JT], f32)
            nc.sync.dma_start(out=b_sb[:], in_=bias[:])

            for _rep in range(repeat):
                col = 0
                while col < R_PAD:
                    ch = min(CHUNK, R_PAD - col)
                    xts = []
                    for kt in range(KT):
                        xtile = io.tile([128, CHUNK], dt_in, tag=f"x{kt}", name=f"x{kt}")
                        nc.sync.dma_start(
                            out=xtile[:, :ch],
                            in_=xT[kt * 128 : (kt + 1) * 128, col : col + ch],
                        )
                        xts.append(xtile)
                    for j2 in range(JT):
                        ot = io.tile([128, CHUNK], f32, tag=f"o{j2}", name=f"o{j2}")
                        for n in range(0, ch, 512):
                            nsz = min(512, ch - n)
                            ps = pp.tile([128, 512], f32, tag="ps", name="ps")
                            for kt in range(KT):
                                nc.tensor.matmul(
                                    ps[:, :nsz],
                                    w_sb[:, kt * DIM + j2 * 128 : kt * DIM + j2 * 128 + 128],
                                    xts[kt][:, n : n + nsz],
                                    start=(kt == 0),
                                    stop=(kt == KT - 1),
                                )
                            nc.vector.tensor_scalar_add(
                                ot[:, n : n + nsz], ps[:, :nsz], b_sb[:, j2 : j2 + 1]
                            )
                        nc.sync.dma_start(
                            out=outT[j2 * 128 : (j2 + 1) * 128, col : col + ch],
                            in_=ot[:, :ch],
                        )
                    col += ch
    nc.compile()
    return nc


def _get_nc(in_dt="float32r", repeat=1):
    key = (in_dt, repeat)
    if key not in _NC_CACHE:
        _NC_CACHE[key] = _build_nc(in_dt, repeat)
    return _NC_CACHE[key]


_TBLK = 256  # row-block size for cache-friendly host transposes


def _transpose_into(dst, src):
    """dst[:, :src.rows] = src.T, blocked for cache locality."""
    rows = src.shape[0]
    for i in range(0, rows, _TBLK):
        j = min(i + _TBLK, rows)
        dst[:, i:j] = src[i:j].T


def _make_core_input(xm, h_sub, c):
    """Per-core feature-major activation block [512, R_PAD] (padded with 0)."""
    lo, hi = c * ROWS_PC, (c + 1) * ROWS_PC
    xT_c = np.empty((2 * DIM, R_PAD), np.float32)
    _transpose_into(xT_c[:DIM], xm[lo:hi])
    _transpose_into(xT_c[DIM:], h_sub[lo:hi])
    xT_c[:, ROWS_PC:] = 0.0
    return xT_c


def _run_device(in_maps):
    _ensure_concourse()
    from concourse.bass_utils import run_bass_kernel_spmd

    nc = _get_nc()
    return run_bass_kernel_spmd(nc, in_maps, list(range(N_CORES))).results


def _copy_rows(dst, src, lo, hi):
    np.copyto(dst[lo:hi], src[lo:hi])


def kernel(h_full, h_sub, W1, b1, W2, b2, old_idxs):
    h_full = np.asarray(h_full, dtype=np.float32)
    h_sub = np.asarray(h_sub, dtype=np.float32)
    W1 = np.asarray(W1, dtype=np.float32)
    W2 = np.asarray(W2, dtype=np.float32)
    b1 = np.asarray(b1, dtype=np.float32)
    b2 = np.asarray(b2, dtype=np.float32)
    idx = np.asarray(old_idxs)

    fast = idx.shape == (M,) and bool(
        np.array_equal(idx, np.arange(M, dtype=idx.dtype))
    )
    xm = h_full[:M] if fast else np.ascontiguousarray(h_full[idx])

    wT = np.ascontiguousarray(np.concatenate([W1.T, W2.T], axis=0), dtype=np.float32)
    bias = np.ascontiguousarray((b1 + b2).astype(np.float32).reshape(JT, 128).T)

    xTs = list(_POOL.map(lambda c: _make_core_input(xm, h_sub, c), range(N_CORES)))
    in_maps = [{"xT": xTs[c], "wT": wT, "bias": bias} for c in range(N_CORES)]

    results = _run_device(in_maps)

    out = np.empty((N, DIM), np.float32)

    def _untranspose_into(dst, src_t):
        rows = dst.shape[0]
        for i in range(0, rows, _TBLK):
            j = min(i + _TBLK, rows)
            dst[i:j] = src_t[:, i:j].T

    if fast:
        def _fill_merged(c):
            _untranspose_into(
                out[c * ROWS_PC : (c + 1) * ROWS_PC], results[c]["outT"][:, :ROWS_PC]
            )

        jobs = [_POOL.submit(_fill_merged, c) for c in range(N_CORES)]
        step = (N - M) // N_CORES
        for c in range(N_CORES):
            lo = M + c * step
            hi = N if c == N_CORES - 1 else M + (c + 1) * step
            jobs.append(_POOL.submit(_copy_rows, out, h_full, lo, hi))
        for j in jobs:
            j.result()
    else:
        merged = np.empty((M, DIM), np.float32)

        def _fill_merged(c):
            _untranspose_into(
                merged[c * ROWS_PC : (c + 1) * ROWS_PC], results[c]["outT"][:, :ROWS_PC]
            )

        list(_POOL.map(_fill_merged, range(N_CORES)))
        np.copyto(out, h_full)
        out[idx] = merged
    return out



# revision 3
# speedup vs baseline: 1.7612x; 1.7612x over previous
"""NodeUnpool kernel for 8 Trainium2 NeuronCores (Bass/Tile, SPMD) — bf16 I/O.

Computation (see nn.Module reference):
    old = h_full[old_idxs]                      # [M, 256] gather
    merged = old @ W1.T + b1 + h_sub @ W2.T + b2
    out = h_full with rows old_idxs replaced by merged

Strategy:
  * old_idxs is arange(M) in this problem (fill="arange"), so the gather and
    scatter are contiguous row slices. A general host-side gather/scatter
    fallback handles any other index pattern.
  * The device work is exactly the merged-row GEMM: X=[old | h_sub] [M,512]
    @ Wc.T + (b1+b2), sharded row-wise across 8 cores (M/8 = 31250 rows each).
  * Activations are fed feature-major (host pre-transpose) so the 512-deep
    contraction lies on SBUF partitions: outT[j,r] = sum_k Wc.T[k,j] * X.T[k,r].
    Weights are the stationary operand; PSUM accumulates 4 k-tiles; the bias is
    added during PSUM->SBUF eviction (per-partition scalar on the DVE).
  * This problem is memory-regime: the kernel is HBM-DMA-bound. Activations,
    weights AND the merged output travel as bfloat16, halving HBM traffic vs
    fp32 (48 MB/core vs 98 MB/core). PSUM accumulation stays fp32; the
    max rel-err vs the fp32 reference is ~2e-3 against a 2e-2 gate.
  * Pass-through rows (h_full[M:]) never touch the device; they are copied on
    the host during output assembly.
"""

import sys
from concurrent.futures import ThreadPoolExecutor

import numpy as np
import ml_dtypes

BF16 = np.dtype(ml_dtypes.bfloat16)

N, M, DIM = 1_000_000, 250_000, 256
N_CORES = 8
ROWS_PC = M // N_CORES  # 31250 merged rows per core
CHUNK = 4096            # columns (rows of X) processed per inner step
R_PAD = 31296           # 31250 padded to a 64 multiple (aligned bf16 DMA lines)
KT = (2 * DIM) // 128   # 4 contraction tiles
JT = DIM // 128         # 2 output-feature blocks

_NC_CACHE = {}
_POOL = ThreadPoolExecutor(max_workers=N_CORES)


def _ensure_concourse():
    try:
        import concourse.bass  # noqa: F401
    except ImportError:  # pragma: no cover
        sys.path.insert(0, "/opt/trn_rl_repo")
        import concourse.bass  # noqa: F401


def _build_nc(in_dt="bfloat16", repeat=1):
    """Build + bacc-compile the per-core Bass program (identical on all cores).

    repeat>1 re-runs the steady-state column loop that many times inside one
    NEFF (idempotent writes) — used by the slope-based HW timing harness.
    """
    _ensure_concourse()
    import concourse.bacc as bacc
    import concourse.tile as tile
    from concourse import mybir

    dt_in = getattr(mybir.dt, in_dt)
    f32 = mybir.dt.float32

    nc = bacc.Bacc("TRN2", target_bir_lowering=False, debug=False)
    xT = nc.dram_tensor("xT", [2 * DIM, R_PAD], dt_in, kind="ExternalInput")
    wT = nc.dram_tensor("wT", [2 * DIM, DIM], dt_in, kind="ExternalInput")
    bias = nc.dram_tensor("bias", [128, JT], f32, kind="ExternalInput")
    outT = nc.dram_tensor("outT", [DIM, R_PAD], dt_in, kind="ExternalOutput")

    with tile.TileContext(nc) as tc:
        with (
            tc.tile_pool(name="wpool", bufs=1) as wpool,
            tc.tile_pool(name="io", bufs=2) as io,
            tc.tile_pool(name="pp", bufs=4, space="PSUM") as pp,
        ):
            w_sb = wpool.tile([128, KT * DIM], dt_in)
            for kt in range(KT):
                nc.sync.dma_start(
                    out=w_sb[:, kt * DIM : (kt + 1) * DIM],
                    in_=wT[kt * 128 : (kt + 1) * 128, :],
                )
            b_sb = wpool.tile([128, JT], f32)
            nc.sync.dma_start(out=b_sb[:], in_=bias[:])

            for _rep in range(repeat):
                col = 0
                while col < R_PAD:
                    ch = min(CHUNK, R_PAD - col)
                    xts = []
                    for kt in range(KT):
                        xtile = io.tile([128, CHUNK], dt_in, tag=f"x{kt}", name=f"x{kt}")
                        nc.sync.dma_start(
                            out=xtile[:, :ch],
                            in_=xT[kt * 128 : (kt + 1) * 128, col : col + ch],
                        )
                        xts.append(xtile)
                    for j2 in range(JT):
                        ot = io.tile([128, CHUNK], dt_in, tag=f"o{j2}", name=f"o{j2}")
                        for n in range(0, ch, 512):
                            nsz = min(512, ch - n)
                            ps = pp.tile([128, 512], f32, tag="ps", name="ps")
                            for kt in range(KT):
                                nc.tensor.matmul(
                                    ps[:, :nsz],
                                    w_sb[:, kt * DIM + j2 * 128 : kt * DIM + j2 * 128 + 128],
                                    xts[kt][:, n : n + nsz],
                                    start=(kt == 0),
                                    stop=(kt == KT - 1),
                                )
                            nc.vector.tensor_scalar_add(
                                ot[:, n : n + nsz], ps[:, :nsz], b_sb[:, j2 : j2 + 1]
                            )
                        nc.sync.dma_start(
                            out=outT[j2 * 128 : (j2 + 1) * 128, col : col + ch],
                            in_=ot[:, :ch],
                        )
                    col += ch
    nc.compile()
    return nc


def _get_nc(in_dt="bfloat16", repeat=1):
    key = (in_dt, repeat)
    if key not in _NC_CACHE:
        _NC_CACHE[key] = _build_nc(in_dt, repeat)
    return _NC_CACHE[key]


_TBLK = 256  # row-block size for cache-friendly host transposes


def _transpose_into(dst, src):
    """dst[:, :src.rows] = src.T, blocked for cache locality."""
    rows = src.shape[0]
    for i in range(0, rows, _TBLK):
        j = min(i + _TBLK, rows)
        dst[:, i:j] = src[i:j].T


def _make_core_input(xm, h_sub, c):
    """Per-core feature-major bf16 activation block [512, R_PAD] (0-padded)."""
    lo, hi = c * ROWS_PC, (c + 1) * ROWS_PC
    xT_c = np.empty((2 * DIM, R_PAD), BF16)
    _transpose_into(xT_c[:DIM], np.ascontiguousarray(xm[lo:hi], dtype=BF16))
    _transpose_into(xT_c[DIM:], np.ascontiguousarray(h_sub[lo:hi], dtype=BF16))
    xT_c[:, ROWS_PC:] = 0.0
    return xT_c


def _run_device(in_maps):
    _ensure_concourse()
    from concourse.bass_utils import run_bass_kernel_spmd

    nc = _get_nc()
    return run_bass_kernel_spmd(nc, in_maps, list(range(N_CORES))).results


def _copy_rows(dst, src, lo, hi):
    np.copyto(dst[lo:hi], src[lo:hi])


def kernel(h_full, h_sub, W1, b1, W2, b2, old_idxs):
    h_full = np.asarray(h_full, dtype=np.float32)
    h_sub = np.asarray(h_sub, dtype=np.float32)
    W1 = np.asarray(W1, dtype=np.float32)
    W2 = np.asarray(W2, dtype=np.float32)
    b1 = np.asarray(b1, dtype=np.float32)
    b2 = np.asarray(b2, dtype=np.float32)
    idx = np.asarray(old_idxs)

    fast = idx.shape == (M,) and bool(
        np.array_equal(idx, np.arange(M, dtype=idx.dtype))
    )
    xm = h_full[:M] if fast else np.ascontiguousarray(h_full[idx])

    wT = np.concatenate([W1.T, W2.T], axis=0).astype(BF16)
    bias = np.ascontiguousarray((b1 + b2).astype(np.float32).reshape(JT, 128).T)

    xTs = list(_POOL.map(lambda c: _make_core_input(xm, h_sub, c), range(N_CORES)))
    in_maps = [{"xT": xTs[c], "wT": wT, "bias": bias} for c in range(N_CORES)]

    results = _run_device(in_maps)

    out = np.empty((N, DIM), np.float32)

    def _untranspose_into(dst, src_t):
        rows = dst.shape[0]
        for i in range(0, rows, _TBLK):
            j = min(i + _TBLK, rows)
            dst[i:j] = src_t[:, i:j].T

    if fast:
        def _fill_merged(c):
            _untranspose_into(
                out[c * ROWS_PC : (c + 1) * ROWS_PC], results[c]["outT"][:, :ROWS_PC]
            )

        jobs = [_POOL.submit(_fill_merged, c) for c in range(N_CORES)]
        step = (N - M) // N_CORES
        for c in range(N_CORES):
            lo = M + c * step
            hi = N if c == N_CORES - 1 else M + (c + 1) * step
            jobs.append(_POOL.submit(_copy_rows, out, h_full, lo, hi))
        for j in jobs:
            j.result()
    else:
        merged = np.empty((M, DIM), np.float32)

        def _fill_merged(c):
            _untranspose_into(
                merged[c * ROWS_PC : (c + 1) * ROWS_PC], results[c]["outT"][:, :ROWS_PC]
            )

        list(_POOL.map(_fill_merged, range(N_CORES)))
        np.copyto(out, h_full)
        out[idx] = merged
    return out


# revision 5
# speedup vs baseline: 2.1660x; 1.2299x over previous
"""NodeUnpool kernel for 8 Trainium2 NeuronCores (Bass/Tile, SPMD) — bf16 I/O.

Computation (see nn.Module reference):
    old = h_full[old_idxs]                      # [M, 256] gather
    merged = old @ W1.T + b1 + h_sub @ W2.T + b2
    out = h_full with rows old_idxs replaced by merged

Strategy:
  * old_idxs is arange(M) in this problem (fill="arange"), so the gather and
    scatter are contiguous row slices. A general host-side gather/scatter
    fallback handles any other index pattern.
  * The device work is exactly the merged-row GEMM: X=[old | h_sub] [M,512]
    @ Wc.T + (b1+b2), sharded row-wise across 8 cores (M/8 = 31250 rows each).
  * Activations are fed feature-major (host pre-transpose) so the 512-deep
    contraction lies on SBUF partitions: outT[j,r] = sum_k Wc.T[k,j] * X.T[k,r].
    Weights are the stationary operand; PSUM accumulates 4 k-tiles; the bias is
    added during PSUM->SBUF eviction (per-partition scalar on the DVE).
  * This problem is memory-regime: the kernel is HBM-DMA-bound. Activations,
    weights AND the merged output travel as bfloat16, halving HBM traffic vs
    fp32 (48 MB/core vs 98 MB/core). PSUM accumulation stays fp32; the
    max rel-err vs the fp32 reference is ~2e-3 against a 2e-2 gate.
  * Pass-through rows (h_full[M:]) never touch the device; they are copied on
    the host during output assembly.
"""

import sys
from concurrent.futures import ThreadPoolExecutor

import numpy as np
import ml_dtypes

BF16 = np.dtype(ml_dtypes.bfloat16)

N, M, DIM = 1_000_000, 250_000, 256
N_CORES = 8
ROWS_PC = M // N_CORES  # 31250 merged rows per core
CHUNK = 4096            # columns (rows of X) processed per inner step
R_PAD = 31296           # 31250 padded to a 64 multiple (aligned bf16 DMA lines)
KT = (2 * DIM) // 128   # 4 contraction tiles
JT = DIM // 128         # 2 output-feature blocks

_NC_CACHE = {}
_POOL = ThreadPoolExecutor(max_workers=N_CORES)


def _ensure_concourse():
    try:
        import concourse.bass  # noqa: F401
    except ImportError:  # pragma: no cover
        sys.path.insert(0, "/opt/trn_rl_repo")
        import concourse.bass  # noqa: F401


def _build_nc(in_dt="bfloat16", repeat=1):
    """Build + bacc-compile the per-core Bass program (identical on all cores).

    repeat>1 re-runs the steady-state column loop that many times inside one
    NEFF (idempotent writes) — used by the slope-based HW timing harness.
    """
    _ensure_concourse()
    import concourse.bacc as bacc
    import concourse.tile as tile
    from concourse import mybir

    dt_in = getattr(mybir.dt, in_dt)
    f32 = mybir.dt.float32

    nc = bacc.Bacc("TRN2", target_bir_lowering=False, debug=False)
    xT = nc.dram_tensor("xT", [2 * DIM, R_PAD], dt_in, kind="ExternalInput")
    wT = nc.dram_tensor("wT", [2 * DIM, DIM], dt_in, kind="ExternalInput")
    bias = nc.dram_tensor("bias", [128, JT], f32, kind="ExternalInput")
    outT = nc.dram_tensor("outT", [DIM, R_PAD], dt_in, kind="ExternalOutput")

    with tile.TileContext(nc) as tc:
        with (
            tc.tile_pool(name="wpool", bufs=1) as wpool,
            tc.tile_pool(name="io", bufs=3) as io,
            tc.tile_pool(name="pp", bufs=4, space="PSUM") as pp,
        ):
            w_sb = wpool.tile([128, KT * DIM], dt_in)
            for kt in range(KT):
                nc.sync.dma_start(
                    out=w_sb[:, kt * DIM : (kt + 1) * DIM],
                    in_=wT[kt * 128 : (kt + 1) * 128, :],
                )
            b_sb = wpool.tile([128, JT], f32)
            nc.sync.dma_start(out=b_sb[:], in_=bias[:])

            for _rep in range(repeat):
                col = 0
                while col < R_PAD:
                    ch = min(CHUNK, R_PAD - col)
                    xts = []
                    for kt in range(KT):
                        xtile = io.tile([128, CHUNK], dt_in, tag=f"x{kt}", name=f"x{kt}")
                        nc.sync.dma_start(
                            out=xtile[:, :ch],
                            in_=xT[kt * 128 : (kt + 1) * 128, col : col + ch],
                        )
                        xts.append(xtile)
                    for j2 in range(JT):
                        ot = io.tile([128, CHUNK], dt_in, tag=f"o{j2}", name=f"o{j2}")
                        for n in range(0, ch, 512):
                            nsz = min(512, ch - n)
                            ps = pp.tile([128, 512], f32, tag="ps", name="ps")
                            for kt in range(KT):
                                nc.tensor.matmul(
                                    ps[:, :nsz],
                                    w_sb[:, kt * DIM + j2 * 128 : kt * DIM + j2 * 128 + 128],
                                    xts[kt][:, n : n + nsz],
                                    start=(kt == 0),
                                    stop=(kt == KT - 1),
                                )
                            nc.vector.tensor_scalar_add(
                                ot[:, n : n + nsz], ps[:, :nsz], b_sb[:, j2 : j2 + 1]
                            )
                        # Output stores go out on the ACT HWDGE ring so their
                        # eviction sem-waits never stall the SP ring that is
                        # streaming input loads (HWDGE is FIFO per engine).
                        nc.scalar.dma_start(
                            out=outT[j2 * 128 : (j2 + 1) * 128, col : col + ch],
                            in_=ot[:, :ch],
                        )
                    col += ch
    nc.compile()
    return nc


def _get_nc(in_dt="bfloat16", repeat=1):
    key = (in_dt, repeat)
    if key not in _NC_CACHE:
        _NC_CACHE[key] = _build_nc(in_dt, repeat)
    return _NC_CACHE[key]


_TBLK = 256  # row-block size for cache-friendly host transposes


def _transpose_into(dst, src):
    """dst[:, :src.rows] = src.T, blocked for cache locality."""
    rows = src.shape[0]
    for i in range(0, rows, _TBLK):
        j = min(i + _TBLK, rows)
        dst[:, i:j] = src[i:j].T


def _make_core_input(xm, h_sub, c):
    """Per-core feature-major bf16 activation block [512, R_PAD] (0-padded)."""
    lo, hi = c * ROWS_PC, (c + 1) * ROWS_PC
    xT_c = np.empty((2 * DIM, R_PAD), BF16)
    _transpose_into(xT_c[:DIM], np.ascontiguousarray(xm[lo:hi], dtype=BF16))
    _transpose_into(xT_c[DIM:], np.ascontiguousarray(h_sub[lo:hi], dtype=BF16))
    xT_c[:, ROWS_PC:] = 0.0
    return xT_c


def _run_device(in_maps):
    _ensure_concourse()
    from concourse.bass_utils import run_bass_kernel_spmd

    nc = _get_nc()
    return run_bass_kernel_spmd(nc, in_maps, list(range(N_CORES))).results


def _copy_rows(dst, src, lo, hi):
    np.copyto(dst[lo:hi], src[lo:hi])


def kernel(h_full, h_sub, W1, b1, W2, b2, old_idxs):
    h_full = np.asarray(h_full, dtype=np.float32)
    h_sub = np.asarray(h_sub, dtype=np.float32)
    W1 = np.asarray(W1, dtype=np.float32)
    W2 = np.asarray(W2, dtype=np.float32)
    b1 = np.asarray(b1, dtype=np.float32)
    b2 = np.asarray(b2, dtype=np.float32)
    idx = np.asarray(old_idxs)

    fast = idx.shape == (M,) and bool(
        np.array_equal(idx, np.arange(M, dtype=idx.dtype))
    )
    xm = h_full[:M] if fast else np.ascontiguousarray(h_full[idx])

    wT = np.concatenate([W1.T, W2.T], axis=0).astype(BF16)
    bias = np.ascontiguousarray((b1 + b2).astype(np.float32).reshape(JT, 128).T)

    xTs = list(_POOL.map(lambda c: _make_core_input(xm, h_sub, c), range(N_CORES)))
    in_maps = [{"xT": xTs[c], "wT": wT, "bias": bias} for c in range(N_CORES)]

    results = _run_device(in_maps)

    out = np.empty((N, DIM), np.float32)

    def _untranspose_into(dst, src_t):
        rows = dst.shape[0]
        for i in range(0, rows, _TBLK):
            j = min(i + _TBLK, rows)
            dst[i:j] = src_t[:, i:j].T

    if fast:
        def _fill_merged(c):
            _untranspose_into(
                out[c * ROWS_PC : (c + 1) * ROWS_PC], results[c]["outT"][:, :ROWS_PC]
            )

        jobs = [_POOL.submit(_fill_merged, c) for c in range(N_CORES)]
        step = (N - M) // N_CORES
        for c in range(N_CORES):
            lo = M + c * step
            hi = N if c == N_CORES - 1 else M + (c + 1) * step
            jobs.append(_POOL.submit(_copy_rows, out, h_full, lo, hi))
        for j in jobs:
            j.result()
    else:
        merged = np.empty((M, DIM), np.float32)

        def _fill_merged(c):
            _untranspose_into(
                merged[c * ROWS_PC : (c + 1) * ROWS_PC], results[c]["outT"][:, :ROWS_PC]
            )

        list(_POOL.map(_fill_merged, range(N_CORES)))
        np.copyto(out, h_full)
        out[idx] = merged
    return out


# revision 8
# speedup vs baseline: 2.3569x; 1.0881x over previous
"""NodeUnpool kernel for 8 Trainium2 NeuronCores (Bass/Tile, SPMD) — bf16 I/O.

Computation (see nn.Module reference):
    old = h_full[old_idxs]                      # [M, 256] gather
    merged = old @ W1.T + b1 + h_sub @ W2.T + b2
    out = h_full with rows old_idxs replaced by merged

Strategy:
  * old_idxs is arange(M) in this problem (fill="arange"), so the gather and
    scatter are contiguous row slices. A general host-side gather/scatter
    fallback handles any other index pattern.
  * The device work is exactly the merged-row GEMM: X=[old | h_sub] [M,512]
    @ Wc.T + (b1+b2), sharded row-wise across 8 cores (M/8 = 31250 rows each).
  * Activations are fed feature-major (host pre-transpose) so the 512-deep
    contraction lies on SBUF partitions: outT[j,r] = sum_k Wc.T[k,j] * X.T[k,r].
    Weights are the stationary operand; PSUM accumulates 4 k-tiles; the bias is
    added during PSUM->SBUF eviction (per-partition scalar on the DVE).
  * This problem is memory-regime: the kernel is HBM-DMA-bound. Activations,
    weights AND the merged output travel as bfloat16, halving HBM traffic vs
    fp32 (48 MB/core vs 98 MB/core). PSUM accumulation stays fp32; the
    max rel-err vs the fp32 reference is 3.8e-3 against a 2e-2 gate.
  * Pass-through rows (h_full[M:]) never touch the device; they are copied on
    the host during output assembly.
"""

import sys
from concurrent.futures import ThreadPoolExecutor

import numpy as np
import ml_dtypes

BF16 = np.dtype(ml_dtypes.bfloat16)

N, M, DIM = 1_000_000, 250_000, 256
N_CORES = 8
ROWS_PC = M // N_CORES  # 31250 merged rows per core
CHUNK = 4096            # columns (rows of X) processed per inner step
R_PAD = 31296           # 31250 padded to a 64 multiple (aligned bf16 DMA lines)
KT = (2 * DIM) // 128   # 4 contraction tiles
JT = DIM // 128         # 2 output-feature blocks
RAMP = [512, 1024, 2048]  # small chunks at both ends: early PE start, short drain


def _chunk_schedule():
    """Column-chunk sizes: ramp up, steady CHUNK-sized body, ramp down.

    One-shot latency = pipeline fill + steady loop + drain. Small leading
    chunks let the first matmul start after ~0.5 MB of DMA instead of 4 MB;
    small trailing chunks shrink the post-last-matmul store tail.
    """
    body = R_PAD - 2 * sum(RAMP)
    n_full, rem = divmod(body, CHUNK)
    return RAMP + [CHUNK] * n_full + ([rem] if rem else []) + RAMP[::-1]

_NC_CACHE = {}
_POOL = ThreadPoolExecutor(max_workers=N_CORES)


def _ensure_concourse():
    try:
        import concourse.bass  # noqa: F401
    except ImportError:  # pragma: no cover
        sys.path.insert(0, "/opt/trn_rl_repo")
        import concourse.bass  # noqa: F401


def _build_nc(in_dt="bfloat16", repeat=1):
    """Build + bacc-compile the per-core Bass program (identical on all cores).

    repeat>1 re-runs the steady-state column loop that many times inside one
    NEFF (idempotent writes) — used by the slope-based HW timing harness.
    """
    _ensure_concourse()
    import concourse.bacc as bacc
    import concourse.tile as tile
    from concourse import mybir

    dt_in = getattr(mybir.dt, in_dt)
    f32 = mybir.dt.float32

    nc = bacc.Bacc("TRN2", target_bir_lowering=False, debug=False)
    xT = nc.dram_tensor("xT", [2 * DIM, R_PAD], dt_in, kind="ExternalInput")
    wT = nc.dram_tensor("wT", [2 * DIM, DIM], dt_in, kind="ExternalInput")
    bias = nc.dram_tensor("bias", [128, JT], f32, kind="ExternalInput")
    outT = nc.dram_tensor("outT", [DIM, R_PAD], dt_in, kind="ExternalOutput")

    with tile.TileContext(nc) as tc:
        with (
            tc.tile_pool(name="wpool", bufs=1) as wpool,
            tc.tile_pool(name="io", bufs=3) as io,
            tc.tile_pool(name="pp", bufs=4, space="PSUM") as pp,
        ):
            w_sb = wpool.tile([128, KT * DIM], dt_in)
            for kt in range(KT):
                nc.sync.dma_start(
                    out=w_sb[:, kt * DIM : (kt + 1) * DIM],
                    in_=wT[kt * 128 : (kt + 1) * 128, :],
                )
            b_sb = wpool.tile([128, JT], f32)
            nc.sync.dma_start(out=b_sb[:], in_=bias[:])

            for _rep in range(repeat):
                col = 0
                for ch in _chunk_schedule():
                    xts = []
                    for kt in range(KT):
                        xtile = io.tile([128, CHUNK], dt_in, tag=f"x{kt}", name=f"x{kt}")
                        nc.sync.dma_start(
                            out=xtile[:, :ch],
                            in_=xT[kt * 128 : (kt + 1) * 128, col : col + ch],
                        )
                        xts.append(xtile)
                    for j2 in range(JT):
                        ot = io.tile([128, CHUNK], dt_in, tag=f"o{j2}", name=f"o{j2}")
                        for n in range(0, ch, 512):
                            nsz = min(512, ch - n)
                            ps = pp.tile([128, 512], f32, tag="ps", name="ps")
                            for kt in range(KT):
                                nc.tensor.matmul(
                                    ps[:, :nsz],
                                    w_sb[:, kt * DIM + j2 * 128 : kt * DIM + j2 * 128 + 128],
                                    xts[kt][:, n : n + nsz],
                                    start=(kt == 0),
                                    stop=(kt == KT - 1),
                                )
                            nc.vector.tensor_scalar_add(
                                ot[:, n : n + nsz], ps[:, :nsz], b_sb[:, j2 : j2 + 1]
                            )
                        # Output stores go out on the ACT HWDGE ring so their
                        # eviction sem-waits never stall the SP ring that is
                        # streaming input loads (HWDGE is FIFO per engine).
                        nc.scalar.dma_start(
                            out=outT[j2 * 128 : (j2 + 1) * 128, col : col + ch],
                            in_=ot[:, :ch],
                        )
                    col += ch
    nc.compile()
    return nc


def _get_nc(in_dt="bfloat16", repeat=1):
    key = (in_dt, repeat)
    if key not in _NC_CACHE:
        _NC_CACHE[key] = _build_nc(in_dt, repeat)
    return _NC_CACHE[key]


_TBLK = 256  # row-block size for cache-friendly host transposes


def _transpose_into(dst, src):
    """dst[:, :src.rows] = src.T, blocked for cache locality."""
    rows = src.shape[0]
    for i in range(0, rows, _TBLK):
        j = min(i + _TBLK, rows)
        dst[:, i:j] = src[i:j].T


def _make_core_input(xm, h_sub, c):
    """Per-core feature-major bf16 activation block [512, R_PAD] (0-padded)."""
    lo, hi = c * ROWS_PC, (c + 1) * ROWS_PC
    xT_c = np.empty((2 * DIM, R_PAD), BF16)
    _transpose_into(xT_c[:DIM], np.ascontiguousarray(xm[lo:hi], dtype=BF16))
    _transpose_into(xT_c[DIM:], np.ascontiguousarray(h_sub[lo:hi], dtype=BF16))
    xT_c[:, ROWS_PC:] = 0.0
    return xT_c


def _run_device(in_maps):
    _ensure_concourse()
    from concourse.bass_utils import run_bass_kernel_spmd

    nc = _get_nc()
    return run_bass_kernel_spmd(nc, in_maps, list(range(N_CORES))).results


def _copy_rows(dst, src, lo, hi):
    np.copyto(dst[lo:hi], src[lo:hi])


def kernel(h_full, h_sub, W1, b1, W2, b2, old_idxs):
    h_full = np.asarray(h_full, dtype=np.float32)
    h_sub = np.asarray(h_sub, dtype=np.float32)
    W1 = np.asarray(W1, dtype=np.float32)
    W2 = np.asarray(W2, dtype=np.float32)
    b1 = np.asarray(b1, dtype=np.float32)
    b2 = np.asarray(b2, dtype=np.float32)
    idx = np.asarray(old_idxs)

    fast = idx.shape == (M,) and bool(
        np.array_equal(idx, np.arange(M, dtype=idx.dtype))
    )
    xm = h_full[:M] if fast else np.ascontiguousarray(h_full[idx])

    wT = np.concatenate([W1.T, W2.T], axis=0).astype(BF16)
    bias = np.ascontiguousarray((b1 + b2).astype(np.float32).reshape(JT, 128).T)

    xTs = list(_POOL.map(lambda c: _make_core_input(xm, h_sub, c), range(N_CORES)))
    in_maps = [{"xT": xTs[c], "wT": wT, "bias": bias} for c in range(N_CORES)]

    results = _run_device(in_maps)

    out = np.empty((N, DIM), np.float32)

    def _untranspose_into(dst, src_t):
        rows = dst.shape[0]
        for i in range(0, rows, _TBLK):
            j = min(i + _TBLK, rows)
            dst[i:j] = src_t[:, i:j].T

    if fast:
        def _fill_merged(c):
            _untranspose_into(
                out[c * ROWS_PC : (c + 1) * ROWS_PC], results[c]["outT"][:, :ROWS_PC]
            )

        jobs = [_POOL.submit(_fill_merged, c) for c in range(N_CORES)]
        step = (N - M) // N_CORES
        for c in range(N_CORES):
            lo = M + c * step
            hi = N if c == N_CORES - 1 else M + (c + 1) * step
            jobs.append(_POOL.submit(_copy_rows, out, h_full, lo, hi))
        for j in jobs:
            j.result()
    else:
        merged = np.empty((M, DIM), np.float32)

        def _fill_merged(c):
            _untranspose_into(
                merged[c * ROWS_PC : (c + 1) * ROWS_PC], results[c]["outT"][:, :ROWS_PC]
            )

        list(_POOL.map(_fill_merged, range(N_CORES)))
        np.copyto(out, h_full)
        out[idx] = merged
    return out


# revision 10
# speedup vs baseline: 2.5140x; 1.0667x over previous
"""NodeUnpool kernel for 8 Trainium2 NeuronCores (Bass/Tile, SPMD) — bf16 I/O.

Computation (see nn.Module reference):
    old = h_full[old_idxs]                      # [M, 256] gather
    merged = old @ W1.T + b1 + h_sub @ W2.T + b2
    out = h_full with rows old_idxs replaced by merged

Strategy:
  * old_idxs is arange(M) in this problem (fill="arange"), so the gather and
    scatter are contiguous row slices. A general host-side gather/scatter
    fallback handles any other index pattern.
  * The device work is exactly the merged-row GEMM: X=[old | h_sub] [M,512]
    @ Wc.T + (b1+b2), sharded row-wise across 8 cores (M/8 = 31250 rows each).
  * Activations are fed feature-major (host pre-transpose) so the 512-deep
    contraction lies on SBUF partitions: outT[j,r] = sum_k Wc.T[k,j] * X.T[k,r].
    Weights are the stationary operand; PSUM accumulates 4 k-tiles; the bias is
    added during PSUM->SBUF eviction (per-partition scalar on the DVE).
  * This problem is memory-regime: the kernel is HBM-DMA-bound. Activations,
    weights AND the merged output travel as bfloat16, halving HBM traffic vs
    fp32 (48 MB/core vs 98 MB/core). PSUM accumulation stays fp32; the
    max rel-err vs the fp32 reference is 3.8e-3 against a 2e-2 gate.
  * Pass-through rows (h_full[M:]) never touch the device; they are copied on
    the host during output assembly.
"""

import sys
from concurrent.futures import ThreadPoolExecutor

import numpy as np
import ml_dtypes

BF16 = np.dtype(ml_dtypes.bfloat16)

N, M, DIM = 1_000_000, 250_000, 256
N_CORES = 8
ROWS_PC = M // N_CORES  # 31250 merged rows per core
CHUNK = 4096            # columns (rows of X) processed per inner step
R_PAD = 31296           # 31250 padded to a 64 multiple (aligned bf16 DMA lines)
KT = (2 * DIM) // 128   # 4 contraction tiles
JT = DIM // 128         # 2 output-feature blocks
RAMP = [512, 1024, 2048]  # small chunks at both ends: early PE start, short drain


def _chunk_schedule():
    """Column-chunk sizes: ramp up, steady CHUNK-sized body, ramp down.

    One-shot latency = pipeline fill + steady loop + drain. Small leading
    chunks let the first matmul start after ~0.5 MB of DMA instead of 4 MB;
    small trailing chunks shrink the post-last-matmul store tail.
    """
    body = R_PAD - 2 * sum(RAMP)
    n_full, rem = divmod(body, CHUNK)
    return RAMP + [CHUNK] * n_full + ([rem] if rem else []) + RAMP[::-1]

_NC_CACHE = {}
_POOL = ThreadPoolExecutor(max_workers=N_CORES)


def _ensure_concourse():
    try:
        import concourse.bass  # noqa: F401
    except ImportError:  # pragma: no cover
        sys.path.insert(0, "/opt/trn_rl_repo")
        import concourse.bass  # noqa: F401


def _build_nc(in_dt="bfloat16", repeat=1):
    """Build + bacc-compile the per-core Bass program (identical on all cores).

    repeat>1 re-runs the steady-state column loop that many times inside one
    NEFF (idempotent writes) — used by the slope-based HW timing harness.
    """
    _ensure_concourse()
    import concourse.bacc as bacc
    import concourse.tile as tile
    from concourse import mybir

    dt_in = getattr(mybir.dt, in_dt)
    f32 = mybir.dt.float32

    nc = bacc.Bacc("TRN2", target_bir_lowering=False, debug=False)
    xT = nc.dram_tensor("xT", [2 * DIM, R_PAD], dt_in, kind="ExternalInput")
    wT = nc.dram_tensor("wT", [2 * DIM, DIM], dt_in, kind="ExternalInput")
    bias = nc.dram_tensor("bias", [128, JT], f32, kind="ExternalInput")
    outT = nc.dram_tensor("outT", [DIM, R_PAD], dt_in, kind="ExternalOutput")

    with tile.TileContext(nc) as tc:
        with (
            tc.tile_pool(name="wpool", bufs=1) as wpool,
            tc.tile_pool(name="io", bufs=3) as io,
            tc.tile_pool(name="pp", bufs=4, space="PSUM") as pp,
        ):
            w_sb = wpool.tile([128, KT * DIM], dt_in)
            for kt in range(KT):
                nc.sync.dma_start(
                    out=w_sb[:, kt * DIM : (kt + 1) * DIM],
                    in_=wT[kt * 128 : (kt + 1) * 128, :],
                )
            b_sb = wpool.tile([128, JT], f32)
            nc.sync.dma_start(out=b_sb[:], in_=bias[:])

            for _rep in range(repeat):
                col = 0
                for ch in _chunk_schedule():
                    xts = []
                    for kt in range(KT):
                        xtile = io.tile([128, CHUNK], dt_in, tag=f"x{kt}", name=f"x{kt}")
                        nc.sync.dma_start(
                            out=xtile[:, :ch],
                            in_=xT[kt * 128 : (kt + 1) * 128, col : col + ch],
                        )
                        xts.append(xtile)
                    for j2 in range(JT):
                        ot = io.tile([128, CHUNK], dt_in, tag=f"o{j2}", name=f"o{j2}")
                        for n in range(0, ch, 512):
                            nsz = min(512, ch - n)
                            ps = pp.tile([128, 512], f32, tag="ps", name="ps")
                            for kt in range(KT):
                                nc.tensor.matmul(
                                    ps[:, :nsz],
                                    w_sb[:, kt * DIM + j2 * 128 : kt * DIM + j2 * 128 + 128],
                                    xts[kt][:, n : n + nsz],
                                    start=(kt == 0),
                                    stop=(kt == KT - 1),
                                )
                            nc.vector.tensor_scalar_add(
                                ot[:, n : n + nsz], ps[:, :nsz], b_sb[:, j2 : j2 + 1]
                            )
                        # Output stores go out on the ACT HWDGE ring so their
                        # eviction sem-waits never stall the SP ring that is
                        # streaming input loads (HWDGE is FIFO per engine).
                        nc.scalar.dma_start(
                            out=outT[j2 * 128 : (j2 + 1) * 128, col : col + ch],
                            in_=ot[:, :ch],
                        )
                    col += ch
    nc.compile()
    return nc


def _get_nc(in_dt="bfloat16", repeat=1):
    key = (in_dt, repeat)
    if key not in _NC_CACHE:
        _NC_CACHE[key] = _build_nc(in_dt, repeat)
    return _NC_CACHE[key]


_TBLK = 256  # row-block size for cache-friendly host transposes


def _transpose_into(dst, src):
    """dst[:, :src.rows] = src.T, blocked for cache locality."""
    rows = src.shape[0]
    for i in range(0, rows, _TBLK):
        j = min(i + _TBLK, rows)
        dst[:, i:j] = src[i:j].T


def _make_core_input(xm, h_sub, c):
    """Per-core feature-major bf16 activation block [512, R_PAD] (0-padded)."""
    lo, hi = c * ROWS_PC, (c + 1) * ROWS_PC
    xT_c = np.empty((2 * DIM, R_PAD), BF16)
    _transpose_into(xT_c[:DIM], np.ascontiguousarray(xm[lo:hi], dtype=BF16))
    _transpose_into(xT_c[DIM:], np.ascontiguousarray(h_sub[lo:hi], dtype=BF16))
    xT_c[:, ROWS_PC:] = 0.0
    return xT_c


def _run_device(in_maps):
    _ensure_concourse()
    from concourse.bass_utils import run_bass_kernel_spmd

    nc = _get_nc()
    return run_bass_kernel_spmd(nc, in_maps, list(range(N_CORES))).results


def _copy_rows(dst, src, lo, hi):
    np.copyto(dst[lo:hi], src[lo:hi])


def kernel(h_full, h_sub, W1, b1, W2, b2, old_idxs):
    h_full = np.asarray(h_full, dtype=np.float32)
    h_sub = np.asarray(h_sub, dtype=np.float32)
    W1 = np.asarray(W1, dtype=np.float32)
    W2 = np.asarray(W2, dtype=np.float32)
    b1 = np.asarray(b1, dtype=np.float32)
    b2 = np.asarray(b2, dtype=np.float32)
    idx = np.asarray(old_idxs)

    fast = idx.shape == (M,) and bool(
        np.array_equal(idx, np.arange(M, dtype=idx.dtype))
    )
    xm = h_full[:M] if fast else np.ascontiguousarray(h_full[idx])

    wT = np.concatenate([W1.T, W2.T], axis=0).astype(BF16)
    bias = np.ascontiguousarray((b1 + b2).astype(np.float32).reshape(JT, 128).T)

    xTs = list(_POOL.map(lambda c: _make_core_input(xm, h_sub, c), range(N_CORES)))
    in_maps = [{"xT": xTs[c], "wT": wT, "bias": bias} for c in range(N_CORES)]

    results = _run_device(in_maps)

    out = np.empty((N, DIM), np.float32)

    def _untranspose_into(dst, src_t):
        rows = dst.shape[0]
        for i in range(0, rows, _TBLK):
            j = min(i + _TBLK, rows)
            dst[i:j] = src_t[:, i:j].T

    if fast:
        def _fill_merged(c):
            _untranspose_into(
                out[c * ROWS_PC : (c + 1) * ROWS_PC], results[c]["outT"][:, :ROWS_PC]
            )

        jobs = [_POOL.submit(_fill_merged, c) for c in range(N_CORES)]
        step = (N - M) // N_CORES
        for c in range(N_CORES):
            lo = M + c * step
            hi = N if c == N_CORES - 1 else M + (c + 1) * step
            jobs.append(_POOL.submit(_copy_rows, out, h_full, lo, hi))
        for j in jobs:
            j.result()
    else:
        merged = np.empty((M, DIM), np.float32)

        def _fill_merged(c):
            _untranspose_into(
                merged[c * ROWS_PC : (c + 1) * ROWS_PC], results[c]["outT"][:, :ROWS_PC]
            )

        list(_POOL.map(_fill_merged, range(N_CORES)))
        np.copyto(out, h_full)
        out[idx] = merged
    return out
